# revision 10
# baseline (speedup 1.0000x reference)
"""Trainium2 Bass kernel for the CellularAutomata step (dense_cnn).

Math (per pixel): s = depthwise3x3(wrap_pad(x), [identity, sobel_x, sobel_y]);
h = relu(s @ W1 + b1); out = clip(x + h @ W2 + b2, 0, 1).

Strategy (pure data parallel, batch -> 8 cores, weights replicated):
  - Host: per-core image to channel-major flat layout [16, 258*258] with wrap
    padding (bf16); the whole output (with junk wrap columns) is computed on a
    padded flat grid and the host slices out the valid 256x256 region.
  - The 3x3 perception conv + W1 are folded (host-side) into a [144, 128]
    matrix.  x is loaded 6x-replicated (dy in {0,1} x dx in {0,1,2}) onto
    96 partitions via two overlapping-window DMAs, so the conv is TWO matmuls
    per 512-px chunk: K=96 (dy0+dy1 taps) + K=96 zero-padded (dy2 taps at rhs
    offset +2P).  Both are tile_size (128,128) so the PE never mode-switches
    inside the conv phase.
  - relu(ph + b1) is split: ScalarE handles 1.5 chunk-pairs, VectorE the rest,
    so neither engine exceeds the PE's per-block time.
  - MLP2: 4 chunks' dx land in ONE [128, 512] PSUM tile via column-tiled
    matmuls (tile_position (0, 32j)) that run concurrently on the PE array.
  - Residual + b2: host precomputes xr = x + b2 (bf16, same flat layout);
    device stacks 4 chunk strips into [128, 512] and adds with one
    tensor_tensor; clip is one fused tensor_scalar (max 0, min 1).
"""

import numpy as np
from contextlib import ExitStack

import concourse.bass as bass
import concourse.tile as tile
from concourse import bacc, mybir
from concourse.bass_utils import run_bass_kernel_spmd

B, S, C, HID = 8, 256, 16, 128
N_CORES = 8
P = S + 2                    # padded width = 258
FLAT = P * P                 # 66564
CH = 512                     # pixels per chunk
BF = 4                       # chunks per block (shared DMA)
NCHUNK = 130                 # covers all valid padded-flat positions
NB = (NCHUNK + BF - 1) // BF
SPAN = (BF - 1) * CH + CH + 2 * P + 8   # block free extent read by matmuls
XLEN = 144 * CH + 2 * P + 16            # padded flat x length

_CACHE = {}


def _build_program(bf=BF, xx_bufs=4, xs_bufs=4, h_bufs=3, ph_bufs=3,
                   pdx_bufs=2, u_bufs=3, o_bufs=4, repeat=1,
                   act_split=3, probe=()):
    """act_split: out of 4 chunks per block, how many chunk-relus run on the
    scalar engine (the rest run on the vector engine).
    probe: timing-only ablations (break math): 'conv1' single conv matmul,
    'nopdx' skip MLP2 matmuls, 'nopost' skip u/o vector ops, 'noout' skip
    output DMAs, 'noxs' skip residual loads, 'noxx' skip conv input loads."""
    f32 = mybir.dt.float32
    bf16 = mybir.dt.bfloat16
    Relu = mybir.ActivationFunctionType.Relu
    add = mybir.AluOpType.add
    op_max = mybir.AluOpType.max
    op_min = mybir.AluOpType.min

    nc = bacc.Bacc("TRN2", target_bir_lowering=False, debug=False,
                   num_devices=N_CORES)

    xfb = nc.dram_tensor("xfb", [C, XLEN], bf16, kind="ExternalInput").ap()
    xs2 = nc.dram_tensor("xs2", [128, NB * CH], bf16,
                         kind="ExternalInput").ap()
    wa = nc.dram_tensor("wa", [96, HID], bf16, kind="ExternalInput").ap()
    wb = nc.dram_tensor("wb", [96, HID], bf16, kind="ExternalInput").ap()
    w2b = nc.dram_tensor("w2b", [HID, 32], bf16, kind="ExternalInput").ap()
    b1s = nc.dram_tensor("b1s", [HID, 1], f32, kind="ExternalInput").ap()
    out2 = nc.dram_tensor("out2", [128, NB * CH], f32,
                          kind="ExternalOutput").ap()

    with tile.TileContext(nc) as tc, ExitStack() as ctx:
        wpool = ctx.enter_context(tc.tile_pool(name="wts", bufs=1))
        wa_sb = wpool.tile([96, HID], bf16)
        nc.sync.dma_start(wa_sb[:], wa)
        wb_sb = wpool.tile([96, HID], bf16)
        nc.sync.dma_start(wb_sb[:], wb)
        w2_sb = wpool.tile([HID, 32], bf16)
        nc.sync.dma_start(w2_sb[:], w2b)
        b1_sb = wpool.tile([HID, 1], f32)
        nc.sync.dma_start(b1_sb[:], b1s)

        xpool = ctx.enter_context(tc.tile_pool(name="xx", bufs=xx_bufs))
        xspool = ctx.enter_context(tc.tile_pool(name="xs", bufs=xs_bufs))
        hpool = ctx.enter_context(tc.tile_pool(name="h", bufs=h_bufs))
        upool = ctx.enter_context(tc.tile_pool(name="u", bufs=u_bufs))
        opool = ctx.enter_context(tc.tile_pool(name="o", bufs=o_bufs))
        ph_pool = ctx.enter_context(
            tc.tile_pool(name="ph", bufs=ph_bufs, space="PSUM"))
        pdx_pool = ctx.enter_context(
            tc.tile_pool(name="pdx", bufs=pdx_bufs, space="PSUM"))

        nblocks = (NCHUNK + bf - 1) // bf

        rep_cm = tc.For_i(0, repeat, 1) if repeat > 1 else None
        if rep_cm is not None:
            rep_cm.__enter__()

        def load_block(b):
            q0 = b * bf * CH
            xx = xpool.tile([96, SPAN], bf16, tag="xx")
            if 'noxx' not in probe:
                for dy in range(2):
                    base = xfb[:, q0 + dy * P:q0 + dy * P + SPAN]
                    src = bass.AP(tensor=base.tensor, offset=base.offset,
                                  ap=[[1, 3]] + [list(p) for p in base.ap])
                    nc.sync.dma_start(xx[48 * dy:48 * (dy + 1), :], src)
            xs = xspool.tile([128, CH], bf16, tag="xs")
            if 'noxs' not in probe:
                nc.sync.dma_start(xs[:], xs2[:, b * CH:(b + 1) * CH])
            return xx, xs

        xx, xs = load_block(0)
        for b in range(nblocks):
            q0 = b * bf * CH
            nxt = load_block(b + 1) if b + 1 < nblocks else (None, None)

            # conv phase: 2 matmuls per chunk, all tile_size (128,128)
            hs = []
            for sp in range(bf // 2):
                ph = ph_pool.tile([HID, 2 * CH], f32)
                for s2 in range(2):
                    f0 = (2 * sp + s2) * CH
                    one = 'conv1' in probe
                    nc.tensor.matmul(
                        ph[:, s2 * CH:(s2 + 1) * CH],
                        lhsT=wa_sb[:], rhs=xx[:, f0:f0 + CH],
                        start=True, stop=one)
                    if not one:
                        nc.tensor.matmul(
                            ph[:, s2 * CH:(s2 + 1) * CH],
                            lhsT=wb_sb[:],
                            rhs=xx[:, f0 + 2 * P:f0 + 2 * P + CH],
                            start=False, stop=True)
                # relu: chunks 0..act_split-1 on ScalarE, rest on VectorE
                h = hpool.tile([HID, 2 * CH], bf16)
                lo = 2 * sp
                n_act = max(0, min(2, act_split - lo))  # chunks of this pair on ACT
                if n_act == 2:
                    nc.scalar.activation(h[:], ph[:], Relu, bias=b1_sb[:])
                elif n_act == 1:
                    nc.scalar.activation(h[:, 0:CH], ph[:, 0:CH], Relu,
                                         bias=b1_sb[:])
                    nc.vector.tensor_scalar(h[:, CH:2 * CH], ph[:, CH:2 * CH],
                                            b1_sb[:], 0.0, op0=add, op1=op_max)
                else:
                    nc.vector.tensor_scalar(h[:], ph[:], b1_sb[:], 0.0,
                                            op0=add, op1=op_max)
                hs.append(h)

            # MLP2 phase: 4 column-tiled matmuls into one stacked PSUM tile
            pdx = pdx_pool.tile([128, CH], f32)
            if 'nopdx' not in probe:
                for j in range(bf):
                    nc.tensor.matmul(
                        pdx[32 * j:32 * (j + 1), :], lhsT=w2_sb[:],
                        rhs=hs[j // 2][:, (j % 2) * CH:(j % 2 + 1) * CH],
                        start=True, stop=True, tile_position=(0, 32 * j))

            # post-ops: u = pdx + (x + b2); o = min(max(u, 0), 1)
            o = opool.tile([128, CH], f32)
            if 'nopost' not in probe:
                u = upool.tile([128, CH], f32)
                nc.vector.tensor_tensor(u[:], pdx[:], xs[:], op=add)
                nc.vector.tensor_scalar(o[:], u[:], 0.0, 1.0, op0=op_max,
                                        op1=op_min)

            if 'noout' not in probe:
                nc.sync.dma_start(out2[:, b * CH:(b + 1) * CH], o[:])
            xx, xs = nxt

        if rep_cm is not None:
            rep_cm.__exit__(None, None, None)

    nc.compile()
    return nc


def _prep_inputs(x, pk, W1, b1, W2, b2):
    bfdt = mybir.dt.np(mybir.dt.bfloat16)
    # conv+W1 folding: Wfull[dy, dx, ci, hid]
    W1r = W1.reshape(C, 3, HID)
    Wfull = np.einsum("ydk,ckh->ydch", pk, W1r)
    wa = np.ascontiguousarray(Wfull[0:2].reshape(96, HID)).astype(bfdt)
    wb = np.zeros((96, HID), np.float32)
    wb[0:48] = Wfull[2].reshape(48, HID)
    wb = wb.astype(bfdt)
    w2b = np.zeros((HID, 32), np.float32)
    w2b[:, :C] = W2
    w2b = w2b.astype(bfdt)
    b1s = np.ascontiguousarray(b1.reshape(HID, 1)).astype(np.float32)

    in_maps = []
    for c in range(N_CORES):
        xt = np.ascontiguousarray(x[c].transpose(2, 0, 1))      # [C, S, S]
        xt = np.pad(xt, ((0, 0), (1, 1), (1, 1)), mode="wrap")  # [C, 258, 258]
        xflat = np.zeros((C, XLEN), np.float32)
        xflat[:, :FLAT] = xt.reshape(C, FLAT)
        # stacked residual plane: xs2[32j+c, b*CH+f] = x[c, 2048b+P+1+512j+f]+b2
        v = (xflat[:, P + 1:P + 1 + NB * BF * CH] +
             b2.reshape(C, 1)).reshape(C, NB, BF, CH)
        xs2 = np.zeros((128, NB, CH), np.float32)
        for j in range(BF):
            xs2[32 * j:32 * j + C] = v[:, :, j, :]
        in_maps.append({
            "xfb": xflat.astype(bfdt), "xs2": xs2.reshape(128, NB * CH).astype(bfdt),
            "wa": wa, "wb": wb, "w2b": w2b, "b1s": b1s,
        })
    return in_maps


def kernel(x, perception_kernel, W1, b1, W2, b2):
    x = np.asarray(x, dtype=np.float32)
    pk = np.asarray(perception_kernel, dtype=np.float32)
    W1 = np.asarray(W1, dtype=np.float32)
    b1 = np.asarray(b1, dtype=np.float32)
    W2 = np.asarray(W2, dtype=np.float32)
    b2 = np.asarray(b2, dtype=np.float32)

    if "nc" not in _CACHE:
        _CACHE["nc"] = _build_program()
    nc = _CACHE["nc"]

    in_maps = _prep_inputs(x, pk, W1, b1, W2, b2)
    res = run_bass_kernel_spmd(nc, in_maps, list(range(N_CORES)))
    _CACHE["exec_time_ns"] = getattr(res, "exec_time_ns", None)
    _CACHE["trace"] = getattr(res, "instructions_and_trace", None)
    outs = []
    for c in range(N_CORES):
        o2 = res.results[c]["out2"].reshape(128, NB, CH)
        w = np.empty((C, NB, BF, CH), np.float32)
        for j in range(BF):
            w[:, :, j, :] = o2[32 * j:32 * j + C]
        of = np.zeros((C, XLEN), np.float32)
        of[:, P + 1:P + 1 + NB * BF * CH] = w.reshape(C, NB * BF * CH)
        of = of[:, :FLAT].reshape(C, P, P)
        outs.append(of[:, 1:S + 1, 1:S + 1].transpose(1, 2, 0))
    return np.ascontiguousarray(np.stack(outs, axis=0), dtype=np.float32)
